# revision 1
# baseline (speedup 1.0000x reference)
"""Trainium2 Bass kernel for GQA causal self-attention (RMS-norm QK + NTK RoPE + proj).

Sharding: 8 cores = 2 batches x 4 KV-head groups. Each core computes QKV
projections (bf16 matmuls, f32 accumulate), RMS-norm + NTK RoPE + gain, and
causal attention in a transposed-softmax formulation (scores are bounded since
q/k are RMS-normalized, so no max subtraction; softmax row sums come from a
ones-column matmul) for its (batch, kv-group). A single 8-way AllToAll then
redistributes attention outputs so each core holds full features for a disjoint
T/8-row slice of BOTH batches, and computes that slice of the output projection
(Wproj in bf16). Host only transposes/slices/concats - all FLOPs are on device.
"""

import sys

for _p in ("/opt/trn_rl_repo", "/root/.axon_site/_ro/trn_rl_repo"):
    if _p not in sys.path:
        sys.path.append(_p)

import numpy as np
import ml_dtypes

import concourse.bass as bass
import concourse.mybir as mybir
import concourse.tile as tile
from concourse import bacc
from concourse.bass import ts, ds
from concourse.bass_utils import run_bass_kernel_spmd

FP32 = mybir.dt.float32
BF16 = mybir.dt.bfloat16
AF = mybir.ActivationFunctionType
OP = mybir.AluOpType

B, DIM, H, HKV = 2, 2048, 16, 4
D = 128
HALF = D // 2
G = H // HKV  # q heads per kv head (= heads per core)
HL = G  # 4 local q heads
ROPE_BASE = 10000.0
TRAIN_LEN = 1024
EPS = float(np.finfo(np.float32).eps)
NCORES = 8
GRP = 4  # cores per batch group


def build_nc(
    T: int, use_collective: bool = True, phases: int = 4, p1stop: int = 60
) -> bass.Bass:
    """Build the per-core Bass program (same program on all 8 cores)."""
    TB = T // 128  # t-blocks
    CH = min(512, T)  # tq chunk width for attention
    NCH = T // CH
    SUB = CH // 128  # 128-blocks per chunk
    TSL = T // NCORES  # sequence slice (per batch) per core after AllToAll
    MT = min(128, TSL)  # proj output t-block partition size
    NTB4 = TSL // MT
    OCH = 512  # proj output column chunk
    NOCH = DIM // OCH
    FO = DIM // 128  # feature k-tiles
    ISQ = 1.0 / float(np.sqrt(D))

    nc = bacc.Bacc("TRN2", target_bir_lowering=False, debug=False, num_devices=NCORES)

    xT = nc.dram_tensor("xT", [DIM, T], BF16, kind="ExternalInput")
    wq = nc.dram_tensor("wq", [DIM, HL * D], BF16, kind="ExternalInput")
    wkv = nc.dram_tensor("wkv", [DIM, 2 * D], BF16, kind="ExternalInput")
    wp = nc.dram_tensor("wp", [DIM, DIM], BF16, kind="ExternalInput")
    cost = nc.dram_tensor("cost", [T, HALF], FP32, kind="ExternalInput")
    sint = nc.dram_tensor("sint", [T, HALF], FP32, kind="ExternalInput")
    gain = nc.dram_tensor("gain", [128, HL], FP32, kind="ExternalInput")
    trimask = nc.dram_tensor("trimask", [128, 128], BF16, kind="ExternalInput")
    ident = nc.dram_tensor("ident", [128, 128], FP32, kind="ExternalInput")
    out = nc.dram_tensor("out", [B, TSL, DIM], FP32, kind="ExternalOutput")

    rg = [list(range(NCORES))]

    with tile.TileContext(nc) as tc:
        with (
            tc.tile_pool(name="consts", bufs=1) as consts,
            tc.tile_pool(name="persist", bufs=1) as persist,
            tc.tile_pool(name="dram", bufs=1, space="DRAM") as dram,
        ):
            # ---- resident constants / weights ----
            wq_sb = consts.tile([128, FO, HL * D], BF16)
            nc.sync.dma_start(wq_sb[:], wq.ap().rearrange("(fo fi) h -> fi fo h", fi=128))
            wkv_sb = consts.tile([128, FO, 2 * D], BF16)
            nc.sync.dma_start(wkv_sb[:], wkv.ap().rearrange("(fo fi) h -> fi fo h", fi=128))
            cos_sb = consts.tile([128, TB, HALF], FP32)
            nc.sync.dma_start(cos_sb[:], cost.ap().rearrange("(tb p) h -> p tb h", p=128))
            sin_sb = consts.tile([128, TB, HALF], FP32)
            nc.sync.dma_start(sin_sb[:], sint.ap().rearrange("(tb p) h -> p tb h", p=128))
            gain_sb = consts.tile([128, HL], FP32)
            nc.sync.dma_start(gain_sb[:], gain.ap())
            tri_sb = consts.tile([128, 128], BF16)
            nc.sync.dma_start(tri_sb[:], trimask.ap())
            id_sb = consts.tile([128, 128], FP32)
            nc.sync.dma_start(id_sb[:], ident.ap())
            ones_sb = consts.tile([128, 1], BF16)
            nc.vector.memset(ones_sb[:], 1.0)
            eps_sb = consts.tile([128, 1], FP32)
            nc.vector.memset(eps_sb[:], EPS)

            # ---- persistent activation buffers ----
            qT_sb = persist.tile([128, HL, T], BF16)  # q, d-major per head
            kT_sb = persist.tile([128, T], BF16)  # k, d-major
            v_sb = persist.tile([128, TB, D], BF16)  # v, t-major tiles

            a2a_in = [dram.tile([NCORES, D, TSL], BF16, name=f"a2a_in{h}") for h in range(HL)]
            a2a_out = [dram.tile([NCORES, D, TSL], BF16, name=f"a2a_out{h}") for h in range(HL)]

            # =============== Phase 1: QKV + norm + rope + transpose ===============
            with (
                tc.tile_pool(name="p1sb", bufs=2) as p1sb,
                tc.tile_pool(name="p1ps", bufs=2, space="PSUM") as p1ps,
                tc.tile_pool(name="p1tp", bufs=2, space="PSUM") as p1tp,
            ):
                kt_ps = None
                for tb in range(TB):
                    xt = p1sb.tile([128, FO, 128], BF16, tag="xt")
                    nc.sync.dma_start(
                        xt[:], xT.ap().rearrange("(fo fi) t -> fi fo t", fi=128)[:, :, ts(tb, 128)]
                    )
                    q_ps = p1ps.tile([128, HL * D], FP32, tag="q_ps")
                    kv_ps = p1ps.tile([128, 2 * D], FP32, tag="kv_ps")
                    for fo in range(FO):
                        nc.tensor.matmul(
                            q_ps[:], xt[:, fo, :], wq_sb[:, fo, :],
                            start=(fo == 0), stop=(fo == FO - 1),
                        )
                    for fo in range(FO):
                        nc.tensor.matmul(
                            kv_ps[:], xt[:, fo, :], wkv_sb[:, fo, :],
                            start=(fo == 0), stop=(fo == FO - 1),
                        )

                    # stage to SBUF
                    q_sb = p1sb.tile([128, HL * D], FP32, tag="q_sb")
                    nc.scalar.copy(q_sb[:], q_ps[:])
                    k_sb = p1sb.tile([128, D], FP32, tag="k_sb")
                    nc.vector.tensor_copy(k_sb[:], kv_ps[:, :D])
                    nc.vector.tensor_copy(v_sb[:, tb, :], kv_ps[:, D:])

                    if p1stop < 20:
                        continue
                    # rms stats: rms = sqrt(sumsq/D + eps) per head (q: 0..3, k: 4)
                    sumsq = p1sb.tile([128, HL + 1], FP32, tag="sumsq")
                    scrq = p1sb.tile([128, HL * D], FP32, tag="scrq")
                    nc.scalar.square(scrq[:], q_sb[:])
                    scrk = p1sb.tile([128, D], FP32, tag="scrk")
                    nc.scalar.square(scrk[:], k_sb[:])
                    nc.vector.tensor_reduce(
                        sumsq[:, :HL],
                        scrq[:].rearrange("p (h d) -> p h d", d=D),
                        mybir.AxisListType.X, OP.add,
                    )
                    nc.vector.tensor_reduce(
                        sumsq[:, HL : HL + 1], scrk[:], mybir.AxisListType.X, OP.add
                    )
                    rms = p1sb.tile([128, HL + 1], FP32, tag="rms")
                    if p1stop < 21:
                        continue
                    nc.scalar.activation(
                        rms[:], sumsq[:], AF.Sqrt, bias=eps_sb[:], scale=1.0 / D
                    )
                    scl = p1sb.tile([128, HL + 1], FP32, tag="scl")
                    if p1stop < 22:
                        continue
                    nc.vector.reciprocal(scl[:], rms[:])
                    sclg = p1sb.tile([128, HL], FP32, tag="sclg")
                    if p1stop < 23:
                        continue
                    nc.vector.tensor_mul(sclg[:], scl[:, :HL], gain_sb[:])

                    if p1stop < 30:
                        continue
                    # rope on raw q/k (norm scale applied after; it commutes)
                    q4 = q_sb[:].rearrange("p (h two half) -> p h two half", two=2, half=HALF)
                    cos_b = cos_sb[:, tb, None, None, :].to_broadcast([128, HL, 2, HALF])
                    sin_b = sin_sb[:, tb, None, None, :].to_broadcast([128, HL, 2, HALF])
                    qa = p1sb.tile([128, HL, 2, HALF], FP32, tag="qa")
                    qb = p1sb.tile([128, HL, 2, HALF], FP32, tag="qb")
                    nc.vector.tensor_mul(qa[:], q4, cos_b)
                    nc.vector.tensor_mul(qb[:], q4, sin_b)
                    q_rot = p1sb.tile([128, HL, 2, HALF], FP32, tag="q_rot")
                    nc.vector.tensor_add(q_rot[:, :, 0, :], qa[:, :, 0, :], qb[:, :, 1, :])
                    nc.vector.tensor_sub(q_rot[:, :, 1, :], qa[:, :, 1, :], qb[:, :, 0, :])

                    if p1stop < 40:
                        continue
                    k2 = k_sb[:].rearrange("p (two half) -> p two half", two=2)
                    cos_k = cos_sb[:, tb, None, :].to_broadcast([128, 2, HALF])
                    sin_k = sin_sb[:, tb, None, :].to_broadcast([128, 2, HALF])
                    ka = p1sb.tile([128, 2, HALF], FP32, tag="ka")
                    kb = p1sb.tile([128, 2, HALF], FP32, tag="kb")
                    nc.gpsimd.tensor_mul(ka[:], k2, cos_k)
                    nc.gpsimd.tensor_mul(kb[:], k2, sin_k)
                    k_rot = p1sb.tile([128, 2, HALF], FP32, tag="k_rot")
                    nc.gpsimd.tensor_add(k_rot[:, 0, :], ka[:, 0, :], kb[:, 1, :])
                    nc.gpsimd.tensor_sub(k_rot[:, 1, :], ka[:, 1, :], kb[:, 0, :])

                    if p1stop < 50:
                        continue
                    # apply rms scale (and gain for q)
                    q_fin = p1sb.tile([128, HL * D], FP32, tag="q_fin")
                    qr2 = q_rot[:].rearrange("p h two half -> p (h two half)")
                    for h in range(HL):
                        nc.scalar.activation(
                            q_fin[:, ts(h, D)], qr2[:, ts(h, D)], AF.Copy,
                            scale=sclg[:, h : h + 1],
                        )
                    k_fin = p1sb.tile([128, D], FP32, tag="k_fin")
                    nc.gpsimd.tensor_scalar_mul(
                        k_fin[:], k_rot[:].rearrange("p two half -> p (two half)"),
                        scl[:, HL : HL + 1],
                    )

                    if p1stop < 60:
                        continue
                    # transpose to d-major
                    qt_ps = p1tp.tile([128, HL * D], FP32, tag="qt_ps")
                    for h in range(HL):
                        nc.tensor.transpose(qt_ps[:, ts(h, D)], q_fin[:, ts(h, D)], id_sb[:])
                    nc.vector.tensor_copy(
                        qT_sb[:, :, ts(tb, 128)],
                        qt_ps[:].rearrange("p (h t) -> p h t", h=HL),
                    )
                    if tb % 4 == 0:
                        kt_ps = p1tp.tile([128, 4 * D], FP32, tag="kt_ps")
                    nc.tensor.transpose(kt_ps[:, ts(tb % 4, D)], k_fin[:], id_sb[:])
                    if tb % 4 == 3 or tb == TB - 1:
                        nb = tb % 4 + 1
                        nc.vector.tensor_copy(
                            kT_sb[:, ds((tb - nb + 1) * 128, nb * 128)], kt_ps[:, : nb * 128]
                        )

            if phases <= 1:
                # debug: dump v (and qT when built) so phase 1 stays live
                nelem = B * TSL * DIM
                with tc.tile_pool(name="dbg", bufs=1) as dbg:
                    dt = dbg.tile([128, nelem // 128], FP32)
                    nc.vector.memset(dt[:], 0.0)
                    nc.vector.tensor_copy(
                        dt[:, : TB * D], v_sb[:].rearrange("p tb d -> p (tb d)")
                    )
                    if p1stop >= 60:
                        nc.vector.tensor_copy(
                            dt[:], qT_sb[:].rearrange("p h t -> p (h t)")[:, : nelem // 128]
                        )
                    nc.sync.dma_start(
                        out.ap()
                        .rearrange("b t o -> (b t o)")
                        .rearrange("(p f) -> p f", p=128),
                        dt[:],
                    )

            # =============== Phase 2: causal attention (transposed softmax) ========
            with (
                tc.tile_pool(name="p2sb", bufs=3) as p2sb,
                tc.tile_pool(name="p2sp", bufs=2, space="PSUM") as p2sp,
                tc.tile_pool(name="p2op", bufs=2, space="PSUM") as p2op,
                tc.tile_pool(name="p2mp", bufs=2, space="PSUM") as p2mp,
            ):
                for h in range(HL if phases >= 2 else 0):
                    for c in range(NCH):
                        nblk = SUB * c + SUB  # total tk blocks for this chunk
                        o_ps = p2op.tile([128, CH], FP32, tag="o_ps")
                        sum_ps = p2mp.tile([1, CH], FP32, tag="sum_ps")
                        for j in range(nblk):
                            dj = j - SUB * c  # >= 0 on diagonal blocks
                            pT = p2sb.tile([128, CH], BF16, tag="pT")
                            s_ps = p2sp.tile([128, CH], FP32, tag="s_ps")
                            if dj < 0:
                                nc.tensor.matmul(
                                    s_ps[:], kT_sb[:, ts(j, 128)],
                                    qT_sb[:, h, ds(c * CH, CH)],
                                    start=True, stop=True,
                                )
                                nc.scalar.activation(pT[:], s_ps[:], AF.Exp, scale=ISQ)
                            else:
                                off = dj * 128
                                w = CH - off
                                nc.tensor.matmul(
                                    s_ps[:, off:CH], kT_sb[:, ts(j, 128)],
                                    qT_sb[:, h, ds(c * CH + off, w)],
                                    start=True, stop=True,
                                )
                                nc.scalar.activation(
                                    pT[:, off:CH], s_ps[:, off:CH], AF.Exp, scale=ISQ
                                )
                                if off > 0:
                                    nc.vector.memset(pT[:, :off], 0.0)
                                nc.vector.tensor_mul(
                                    pT[:, off : off + 128], pT[:, off : off + 128], tri_sb[:]
                                )
                            nc.tensor.matmul(
                                o_ps[:], v_sb[:, j, :], pT[:],
                                start=(j == 0), stop=(j == nblk - 1),
                            )
                            nc.tensor.matmul(
                                sum_ps[:], ones_sb[:], pT[:],
                                start=(j == 0), stop=(j == nblk - 1),
                            )
                        rs = p2sb.tile([1, CH], FP32, tag="rs")
                        nc.vector.reciprocal(rs[:], sum_ps[:])
                        rb = p2sb.tile([128, CH], FP32, tag="rb")
                        nc.gpsimd.partition_broadcast(rb[:], rs[:])
                        stage = p2sb.tile([128, CH], BF16, tag="stage")
                        nc.vector.tensor_mul(stage[:], o_ps[:], rb[:])
                        nsl = CH // TSL
                        for sl in range(nsl):
                            nc.sync.dma_start(
                                a2a_in[h][c * nsl + sl, :, :],
                                stage[:, ts(sl, TSL)],
                            )

            # =============== Phase 3: AllToAll across all 8 cores =============
            if phases < 3:
                pass
            elif use_collective:
                for h in range(HL):
                    nc.gpsimd.collective_compute(
                        "AllToAll", OP.bypass, replica_groups=rg,
                        ins=[a2a_in[h][:]], outs=[a2a_out[h][:]],
                    )
            else:
                for h in range(HL):
                    nc.sync.dma_start(a2a_out[h][:], a2a_in[h][:])

            # =============== Phase 4: output projection (row-sharded) =============
            with (
                tc.tile_pool(name="p4sb", bufs=2) as p4sb,
                tc.tile_pool(name="p4in", bufs=1) as p4in,
                tc.tile_pool(name="p4ps", bufs=4, space="PSUM") as p4ps,
            ):
                pin_sb = p4in.tile([128, B, FO, TSL], BF16)
                for beta in range(B if phases >= 4 else 0):
                    for g in range(GRP):
                        for h in range(HL):
                            nc.sync.dma_start(
                                pin_sb[:, beta, g * HL + h],
                                a2a_out[h][beta * GRP + g],
                            )
                for oc in range(NOCH if phases >= 4 else 0):
                    wp_sb = p4sb.tile([128, FO, OCH], BF16, tag="wp_sb")
                    nc.sync.dma_start(
                        wp_sb[:],
                        wp.ap().rearrange("(fo fi) o -> fi fo o", fi=128)[:, :, ts(oc, OCH)],
                    )
                    for beta in range(B):
                        for tb in range(NTB4):
                            pr_ps = p4ps.tile([MT, OCH], FP32, tag="pr_ps")
                            for fo in range(FO):
                                nc.tensor.matmul(
                                    pr_ps[:], pin_sb[:, beta, fo, ts(tb, MT)],
                                    wp_sb[:, fo, :],
                                    start=(fo == 0), stop=(fo == FO - 1),
                                )
                            o_sb = p4sb.tile([MT, OCH], FP32, tag="o_sb")
                            if tb % 2 == 0:
                                nc.vector.tensor_copy(o_sb[:], pr_ps[:])
                            else:
                                nc.scalar.copy(o_sb[:], pr_ps[:])
                            nc.sync.dma_start(
                                out.ap()[beta, ts(tb, MT), ts(oc, OCH)], o_sb[:]
                            )

    nc.compile()
    return nc


def _rope_tables(T: int):
    if T > TRAIN_LEN:
        scale = T / TRAIN_LEN
        base = ROPE_BASE * scale ** (D / (D - 2))
    else:
        base = ROPE_BASE
    inv_freq = 1.0 / base ** (np.arange(0, D, 2, dtype=np.float32) / D)
    freqs = np.outer(np.arange(T, dtype=np.float32), inv_freq)
    return (
        np.cos(freqs).astype(np.float32),
        np.sin(freqs).astype(np.float32),
    )


def make_in_maps(x, Wq, Wk, Wv, Wproj, q_gain, T: int):
    cos, sin = _rope_tables(T)
    tri = np.triu(np.ones((128, 128), dtype=ml_dtypes.bfloat16))
    ident = np.eye(128, dtype=np.float32)
    wpT = np.ascontiguousarray(Wproj.T).astype(ml_dtypes.bfloat16)
    xTs = [np.ascontiguousarray(x[b].T).astype(ml_dtypes.bfloat16) for b in range(x.shape[0])]
    in_maps = []
    for c in range(NCORES):
        b, g = c // GRP, c % GRP
        wq_c = np.ascontiguousarray(Wq[g * HL * D : (g + 1) * HL * D, :].T).astype(
            ml_dtypes.bfloat16
        )
        wkv_c = np.ascontiguousarray(
            np.concatenate([Wk[g * D : (g + 1) * D, :], Wv[g * D : (g + 1) * D, :]], axis=0).T
        ).astype(ml_dtypes.bfloat16)
        gain_c = np.broadcast_to(
            q_gain[g * HL : (g + 1) * HL][None, :], (128, HL)
        ).astype(np.float32).copy()
        in_maps.append(
            {
                "xT": xTs[b],
                "wq": wq_c,
                "wkv": wkv_c,
                "wp": wpT,
                "cost": cos,
                "sint": sin,
                "gain": gain_c,
                "trimask": tri,
                "ident": ident,
            }
        )
    return in_maps


_NC_CACHE = {}


def run(x, Wq, Wk, Wv, Wproj, q_gain, T=None, use_collective=True, **spmd_kwargs):
    T = T if T is not None else x.shape[1]
    key = (T, use_collective)
    if key not in _NC_CACHE:
        _NC_CACHE[key] = build_nc(T, use_collective)
    nc = _NC_CACHE[key]
    in_maps = make_in_maps(x, Wq, Wk, Wv, Wproj, q_gain, T)
    res = run_bass_kernel_spmd(nc, in_maps, core_ids=list(range(NCORES)), **spmd_kwargs)
    TSL = T // NCORES
    out = np.empty((x.shape[0], T, DIM), dtype=np.float32)
    for c in range(NCORES):
        out[:, c * TSL : (c + 1) * TSL, :] = res.results[c]["out"]
    return out, res


def kernel(x, Wq, Wk, Wv, Wproj, q_gain):
    x = np.asarray(x, dtype=np.float32)
    out, _ = run(
        x,
        np.asarray(Wq, dtype=np.float32),
        np.asarray(Wk, dtype=np.float32),
        np.asarray(Wv, dtype=np.float32),
        np.asarray(Wproj, dtype=np.float32),
        np.asarray(q_gain, dtype=np.float32),
    )
    return out



# revision 13
# speedup vs baseline: 3.8344x; 3.8344x over previous
"""Trainium2 Bass kernel for GQA causal self-attention (RMS-norm QK + NTK RoPE + proj).

Sharding: 8 cores = 2 batches x 4 KV-head groups. Each core computes QKV
projections (bf16 matmuls, f32 accumulate), RMS-norm + NTK RoPE + gain, and
causal attention in a transposed-softmax formulation (scores are bounded since
q/k are RMS-normalized, so no max subtraction; softmax row sums come from a
ones-column matmul) for its (batch, kv-group). A single 8-way AllToAll then
redistributes attention outputs so each core holds full features for a disjoint
T/8-row slice of BOTH batches, and computes that slice of the output projection
(Wproj in bf16). Host only transposes/slices/concats - all FLOPs are on device.
"""

import sys

for _p in ("/opt/trn_rl_repo", "/root/.axon_site/_ro/trn_rl_repo"):
    if _p not in sys.path:
        sys.path.append(_p)

import numpy as np
import ml_dtypes

import concourse.bass as bass
import concourse.mybir as mybir
import concourse.tile as tile
from concourse import bacc
from concourse.bass import ts, ds
from concourse.bass_utils import run_bass_kernel_spmd

FP32 = mybir.dt.float32
BF16 = mybir.dt.bfloat16
AF = mybir.ActivationFunctionType
OP = mybir.AluOpType

B, DIM, H, HKV = 2, 2048, 16, 4
D = 128
HALF = D // 2
G = H // HKV  # q heads per kv head (= heads per core)
HL = G  # 4 local q heads
ROPE_BASE = 10000.0
TRAIN_LEN = 1024
EPS = float(np.finfo(np.float32).eps)
NCORES = 8
GRP = 4  # cores per batch group


def build_nc(
    T: int,
    use_collective: bool = True,
    phases: int = 4,
    p1stop: int = 60,
    packed_coll: bool = False,
    npack: int | None = None,
) -> bass.Bass:
    """Build the per-core Bass program (same program on all 8 cores).

    npack: number of AllToAll collectives the 4 heads are split over
    (1 = single packed exchange, 4 = one per head). Each fires as soon
    as its head group's attention output is staged.
    """
    if npack is None:
        npack = 1 if packed_coll else HL
    GH = HL // npack  # heads per collective group
    TB = T // 128  # t-blocks
    CH = min(512, T)  # tq chunk width for attention
    NCH = T // CH
    SUB = CH // 128  # 128-blocks per chunk
    TSL = T // NCORES  # sequence slice (per batch) per core after AllToAll
    MT = min(128, TSL)  # proj output t-block partition size
    NTB4 = TSL // MT
    OCH = 512  # proj output column chunk
    NOCH = DIM // OCH
    FO = DIM // 128  # feature k-tiles
    ISQ = 1.0 / float(np.sqrt(D))

    nc = bacc.Bacc("TRN2", target_bir_lowering=False, debug=False, num_devices=NCORES)

    xT = nc.dram_tensor("xT", [DIM, T], BF16, kind="ExternalInput")
    wq = nc.dram_tensor("wq", [DIM, HL * D], BF16, kind="ExternalInput")
    wkv = nc.dram_tensor("wkv", [DIM, 2 * D], BF16, kind="ExternalInput")
    wp = nc.dram_tensor("wp", [DIM, DIM], BF16, kind="ExternalInput")
    cost = nc.dram_tensor("cost", [T, HALF], FP32, kind="ExternalInput")
    sint = nc.dram_tensor("sint", [T, HALF], FP32, kind="ExternalInput")
    gain = nc.dram_tensor("gain", [128, HL], FP32, kind="ExternalInput")
    trimask = nc.dram_tensor("trimask", [128, 128], BF16, kind="ExternalInput")
    ident = nc.dram_tensor("ident", [128, 128], FP32, kind="ExternalInput")
    out = nc.dram_tensor("out", [B, TSL, DIM], FP32, kind="ExternalOutput")

    rg = [list(range(NCORES))]

    with tile.TileContext(nc) as tc:
        with (
            tc.tile_pool(name="consts", bufs=1) as consts,
            tc.tile_pool(name="persist", bufs=1) as persist,
            tc.tile_pool(name="dram", bufs=1, space="DRAM") as dram,
        ):
            # ---- resident constants / weights ----
            wq_sb = consts.tile([128, FO, HL * D], BF16)
            nc.sync.dma_start(wq_sb[:], wq.ap().rearrange("(fo fi) h -> fi fo h", fi=128))
            wkv_sb = consts.tile([128, FO, 2 * D], BF16)
            nc.sync.dma_start(wkv_sb[:], wkv.ap().rearrange("(fo fi) h -> fi fo h", fi=128))
            cos_sb = consts.tile([128, TB, HALF], FP32)
            nc.sync.dma_start(cos_sb[:], cost.ap().rearrange("(tb p) h -> p tb h", p=128))
            sin_sb = consts.tile([128, TB, HALF], FP32)
            nc.sync.dma_start(sin_sb[:], sint.ap().rearrange("(tb p) h -> p tb h", p=128))
            gain_sb = consts.tile([128, HL], FP32)
            nc.sync.dma_start(gain_sb[:], gain.ap())
            tri_sb = consts.tile([128, 128], BF16)
            nc.sync.dma_start(tri_sb[:], trimask.ap())
            id_sb = consts.tile([128, 128], FP32)
            nc.sync.dma_start(id_sb[:], ident.ap())
            ones_sb = consts.tile([128, 1], BF16)
            nc.vector.memset(ones_sb[:], 1.0)
            eps_sb = consts.tile([128, 1], FP32)
            nc.vector.memset(eps_sb[:], EPS)

            # ---- persistent activation buffers ----
            qT_sb = persist.tile([128, HL, T], BF16)  # q, d-major per head
            kT_sb = persist.tile([128, T], BF16)  # k, d-major
            v_sb = persist.tile([128, TB, D], BF16)  # v, t-major tiles

            # one exchange buffer pair per head-group collective
            a2a_gin = [
                dram.tile([NCORES, GH, D, TSL], BF16, name=f"a2a_in{p}")
                for p in range(npack)
            ]
            a2a_gout = [
                dram.tile([NCORES, GH, D, TSL], BF16, name=f"a2a_out{p}")
                for p in range(npack)
            ]
            a2a_in = [a2a_gin[h // GH][:, h % GH] for h in range(HL)]
            a2a_out = [a2a_gout[h // GH][:, h % GH] for h in range(HL)]

            # =============== Phase 1: QKV + norm + rope + transpose ===============
            with (
                tc.tile_pool(name="p1sb", bufs=2) as p1sb,
                tc.tile_pool(name="p1ps", bufs=2, space="PSUM") as p1ps,
                tc.tile_pool(name="p1tp", bufs=2, space="PSUM") as p1tp,
            ):
                kt_ps = None
                for tb in range(TB):
                    xt = p1sb.tile([128, FO, 128], BF16, tag="xt")
                    nc.sync.dma_start(
                        xt[:], xT.ap().rearrange("(fo fi) t -> fi fo t", fi=128)[:, :, ts(tb, 128)]
                    )
                    q_ps = p1ps.tile([128, HL * D], FP32, tag="q_ps")
                    kv_ps = p1ps.tile([128, 2 * D], FP32, tag="kv_ps")
                    for fo in range(FO):
                        nc.tensor.matmul(
                            q_ps[:], xt[:, fo, :], wq_sb[:, fo, :],
                            start=(fo == 0), stop=(fo == FO - 1),
                        )
                    for fo in range(FO):
                        nc.tensor.matmul(
                            kv_ps[:], xt[:, fo, :], wkv_sb[:, fo, :],
                            start=(fo == 0), stop=(fo == FO - 1),
                        )

                    # stage to SBUF (DVE); squares read PSUM directly (ACT)
                    q_sb = p1sb.tile([128, HL * D], FP32, tag="q_sb")
                    nc.vector.tensor_copy(q_sb[:], q_ps[:])
                    k_sb = p1sb.tile([128, D], FP32, tag="k_sb")
                    nc.vector.tensor_copy(k_sb[:], kv_ps[:, :D])
                    nc.vector.tensor_copy(v_sb[:, tb, :], kv_ps[:, D:])

                    # rms stats: rms = sqrt(sumsq/D + eps) per head (q: 0..3, k: 4)
                    sumsq = p1sb.tile([128, HL + 1], FP32, tag="sumsq")
                    scrq = p1sb.tile([128, HL * D], FP32, tag="scrq")
                    nc.scalar.square(scrq[:], q_ps[:])
                    scrk = p1sb.tile([128, D], FP32, tag="scrk")
                    nc.scalar.square(scrk[:], kv_ps[:, :D])
                    nc.vector.tensor_reduce(
                        sumsq[:, :HL],
                        scrq[:].rearrange("p (h d) -> p h d", d=D),
                        mybir.AxisListType.X, OP.add,
                    )
                    nc.vector.tensor_reduce(
                        sumsq[:, HL : HL + 1], scrk[:], mybir.AxisListType.X, OP.add
                    )
                    rms = p1sb.tile([128, HL + 1], FP32, tag="rms")
                    nc.scalar.activation(
                        rms[:], sumsq[:], AF.Sqrt, bias=eps_sb[:], scale=1.0 / D
                    )
                    scl = p1sb.tile([128, HL + 1], FP32, tag="scl")
                    nc.vector.reciprocal(scl[:], rms[:])
                    sclg = p1sb.tile([128, HL], FP32, tag="sclg")
                    nc.vector.tensor_mul(sclg[:], scl[:, :HL], gain_sb[:])

                    # fold rms scale (and gain) into per-head cos/sin tables, so
                    # the rope output is already final — no post-scale pass
                    csg = p1sb.tile([128, HL, HALF], FP32, tag="csg")
                    ssg = p1sb.tile([128, HL, HALF], FP32, tag="ssg")
                    for h in range(HL):
                        nc.gpsimd.tensor_scalar_mul(
                            csg[:, h, :], cos_sb[:, tb, :], sclg[:, h : h + 1]
                        )
                        nc.gpsimd.tensor_scalar_mul(
                            ssg[:, h, :], sin_sb[:, tb, :], sclg[:, h : h + 1]
                        )
                    q4 = q_sb[:].rearrange("p (h two half) -> p h two half", two=2, half=HALF)
                    cos_b = csg[:, :, None, :].to_broadcast([128, HL, 2, HALF])
                    sin_b = ssg[:, :, None, :].to_broadcast([128, HL, 2, HALF])
                    qa = p1sb.tile([128, HL, 2, HALF], FP32, tag="qa")
                    qb = p1sb.tile([128, HL, 2, HALF], FP32, tag="qb")
                    nc.vector.tensor_mul(qa[:], q4, cos_b)
                    nc.vector.tensor_mul(qb[:], q4, sin_b)
                    q_fin = p1sb.tile([128, HL, 2, HALF], FP32, tag="q_fin")
                    nc.vector.tensor_add(q_fin[:, :, 0, :], qa[:, :, 0, :], qb[:, :, 1, :])
                    nc.vector.tensor_sub(q_fin[:, :, 1, :], qa[:, :, 1, :], qb[:, :, 0, :])

                    ck = p1sb.tile([128, HALF], FP32, tag="ck")
                    sk = p1sb.tile([128, HALF], FP32, tag="sk")
                    nc.gpsimd.tensor_scalar_mul(ck[:], cos_sb[:, tb, :], scl[:, HL : HL + 1])
                    nc.gpsimd.tensor_scalar_mul(sk[:], sin_sb[:, tb, :], scl[:, HL : HL + 1])
                    k2 = k_sb[:].rearrange("p (two half) -> p two half", two=2)
                    cos_k = ck[:, None, :].to_broadcast([128, 2, HALF])
                    sin_k = sk[:, None, :].to_broadcast([128, 2, HALF])
                    ka = p1sb.tile([128, 2, HALF], FP32, tag="ka")
                    kb = p1sb.tile([128, 2, HALF], FP32, tag="kb")
                    nc.gpsimd.tensor_mul(ka[:], k2, cos_k)
                    nc.gpsimd.tensor_mul(kb[:], k2, sin_k)
                    k_fin = p1sb.tile([128, 2, HALF], FP32, tag="k_fin")
                    nc.gpsimd.tensor_add(k_fin[:, 0, :], ka[:, 0, :], kb[:, 1, :])
                    nc.gpsimd.tensor_sub(k_fin[:, 1, :], ka[:, 1, :], kb[:, 0, :])

                    # transpose to d-major
                    qf2 = q_fin[:].rearrange("p h two half -> p (h two half)")
                    kf2 = k_fin[:].rearrange("p two half -> p (two half)")
                    qt_ps = p1tp.tile([128, HL * D], FP32, tag="qt_ps")
                    for h in range(HL):
                        nc.tensor.transpose(qt_ps[:, ts(h, D)], qf2[:, ts(h, D)], id_sb[:])
                    nc.vector.tensor_copy(
                        qT_sb[:, :, ts(tb, 128)],
                        qt_ps[:].rearrange("p (h t) -> p h t", h=HL),
                    )
                    if tb % 4 == 0:
                        kt_ps = p1tp.tile([128, 4 * D], FP32, tag="kt_ps")
                    nc.tensor.transpose(kt_ps[:, ts(tb % 4, D)], kf2[:], id_sb[:])
                    if tb % 4 == 3 or tb == TB - 1:
                        nb = tb % 4 + 1
                        nc.vector.tensor_copy(
                            kT_sb[:, ds((tb - nb + 1) * 128, nb * 128)], kt_ps[:, : nb * 128]
                        )

            if phases <= 1:
                # debug: dump v (and qT when built) so phase 1 stays live
                nelem = B * TSL * DIM
                with tc.tile_pool(name="dbg", bufs=1) as dbg:
                    dt = dbg.tile([128, nelem // 128], FP32)
                    nc.vector.memset(dt[:], 0.0)
                    nc.vector.tensor_copy(
                        dt[:, : TB * D], v_sb[:].rearrange("p tb d -> p (tb d)")
                    )
                    if p1stop >= 60:
                        nc.vector.tensor_copy(
                            dt[:], qT_sb[:].rearrange("p h t -> p (h t)")[:, : nelem // 128]
                        )
                    nc.sync.dma_start(
                        out.ap()
                        .rearrange("b t o -> (b t o)")
                        .rearrange("(p f) -> p f", p=128),
                        dt[:],
                    )

            # =============== Phase 2: causal attention (transposed softmax) ========
            with (
                tc.tile_pool(name="p2sb", bufs=4) as p2sb,
                tc.tile_pool(name="p2sp", bufs=4, space="PSUM") as p2sp,
                tc.tile_pool(name="p2op", bufs=2, space="PSUM") as p2op,
                tc.tile_pool(name="p2mp", bufs=2, space="PSUM") as p2mp,
            ):
                for h in range(HL if phases >= 2 else 0):
                    for c in range(NCH):
                        nblk = SUB * c + SUB  # total tk blocks for this chunk
                        o_ps = p2op.tile([128, CH], FP32, tag="o_ps")
                        sum_ps = p2mp.tile([1, CH], FP32, tag="sum_ps")
                        for j in range(nblk):
                            dj = j - SUB * c  # >= 0 on diagonal blocks
                            off = 0 if dj < 0 else dj * 128
                            w = CH - off
                            pT = p2sb.tile([128, CH], BF16, tag="pT")
                            s_ps = p2sp.tile([128, CH], FP32, tag="s_ps")
                            nc.tensor.matmul(
                                s_ps[:, off:CH], kT_sb[:, ts(j, 128)],
                                qT_sb[:, h, ds(c * CH + off, w)],
                                start=True, stop=True,
                            )
                            nc.scalar.activation(
                                pT[:, off:CH], s_ps[:, off:CH], AF.Exp, scale=ISQ
                            )
                            if dj >= 0:
                                nc.vector.tensor_mul(
                                    pT[:, off : off + 128], pT[:, off : off + 128], tri_sb[:]
                                )
                            # columns [:off] of pT are stale garbage from the
                            # recycled slot; o/sum matmuls only read [off:], so
                            # no memset is needed (j==0 is always full width).
                            nc.tensor.matmul(
                                o_ps[:, off:CH], v_sb[:, j, :], pT[:, off:CH],
                                start=(j == 0), stop=(j == nblk - 1),
                            )
                            nc.tensor.matmul(
                                sum_ps[:, off:CH], ones_sb[:], pT[:, off:CH],
                                start=(j == 0), stop=(j == nblk - 1),
                            )
                        rs = p2sb.tile([1, CH], FP32, tag="rs")
                        nc.vector.reciprocal(rs[:], sum_ps[:])
                        rb = p2sb.tile([128, CH], FP32, tag="rb")
                        nc.gpsimd.partition_broadcast(rb[:], rs[:])
                        stage = p2sb.tile([128, CH], BF16, tag="stage")
                        nc.vector.tensor_mul(stage[:], o_ps[:], rb[:])
                        nsl = CH // TSL
                        for sl in range(nsl):
                            nc.sync.dma_start(
                                a2a_in[h][c * nsl + sl, :, :],
                                stage[:, ts(sl, TSL)],
                            )
                    # Phase 3 (interleaved): fire a head group's AllToAll as
                    # soon as its last head is staged, overlapping the rest.
                    if phases >= 3 and use_collective and h % GH == GH - 1:
                        p = h // GH
                        nc.gpsimd.collective_compute(
                            "AllToAll", OP.bypass, replica_groups=rg,
                            ins=[a2a_gin[p][:]], outs=[a2a_gout[p][:]],
                        )

            if phases >= 3 and not use_collective:
                for p in range(npack):
                    nc.sync.dma_start(a2a_gout[p][:], a2a_gin[p][:])

            # =============== Phase 4: output projection (row-sharded) =============
            with (
                tc.tile_pool(name="p4sb", bufs=2) as p4sb,
                tc.tile_pool(name="p4in", bufs=1) as p4in,
                tc.tile_pool(name="p4ps", bufs=4, space="PSUM") as p4ps,
            ):
                pin_sb = p4in.tile([128, B, FO, TSL], BF16)
                # load + accumulate in head-major order so proj matmuls can
                # begin once head 0's AllToAll has landed
                fo_order = [g * HL + h for h in range(HL) for g in range(GRP)]
                for h in range(HL if phases >= 4 else 0):
                    for beta in range(B):
                        for g in range(GRP):
                            nc.sync.dma_start(
                                pin_sb[:, beta, g * HL + h],
                                a2a_out[h][beta * GRP + g],
                            )
                for oc in range(NOCH if phases >= 4 else 0):
                    wp_sb = p4sb.tile([128, FO, OCH], BF16, tag="wp_sb")
                    nc.sync.dma_start(
                        wp_sb[:],
                        wp.ap().rearrange("(fo fi) o -> fi fo o", fi=128)[:, :, ts(oc, OCH)],
                    )
                    for beta in range(B):
                        for tb in range(NTB4):
                            pr_ps = p4ps.tile([MT, OCH], FP32, tag="pr_ps")
                            for i, fo in enumerate(fo_order):
                                nc.tensor.matmul(
                                    pr_ps[:], pin_sb[:, beta, fo, ts(tb, MT)],
                                    wp_sb[:, fo, :],
                                    start=(i == 0), stop=(i == FO - 1),
                                )
                            o_sb = p4sb.tile([MT, OCH], FP32, tag="o_sb")
                            if tb % 2 == 0:
                                nc.vector.tensor_copy(o_sb[:], pr_ps[:])
                            else:
                                nc.scalar.copy(o_sb[:], pr_ps[:])
                            nc.sync.dma_start(
                                out.ap()[beta, ts(tb, MT), ts(oc, OCH)], o_sb[:]
                            )

    nc.compile()
    return nc


def _rope_tables(T: int):
    if T > TRAIN_LEN:
        scale = T / TRAIN_LEN
        base = ROPE_BASE * scale ** (D / (D - 2))
    else:
        base = ROPE_BASE
    inv_freq = 1.0 / base ** (np.arange(0, D, 2, dtype=np.float32) / D)
    freqs = np.outer(np.arange(T, dtype=np.float32), inv_freq)
    return (
        np.cos(freqs).astype(np.float32),
        np.sin(freqs).astype(np.float32),
    )


def make_in_maps(x, Wq, Wk, Wv, Wproj, q_gain, T: int):
    cos, sin = _rope_tables(T)
    tri = np.triu(np.ones((128, 128), dtype=ml_dtypes.bfloat16))
    ident = np.eye(128, dtype=np.float32)
    wpT = np.ascontiguousarray(Wproj.T).astype(ml_dtypes.bfloat16)
    xTs = [np.ascontiguousarray(x[b].T).astype(ml_dtypes.bfloat16) for b in range(x.shape[0])]
    in_maps = []
    for c in range(NCORES):
        b, g = c // GRP, c % GRP
        wq_c = np.ascontiguousarray(Wq[g * HL * D : (g + 1) * HL * D, :].T).astype(
            ml_dtypes.bfloat16
        )
        wkv_c = np.ascontiguousarray(
            np.concatenate([Wk[g * D : (g + 1) * D, :], Wv[g * D : (g + 1) * D, :]], axis=0).T
        ).astype(ml_dtypes.bfloat16)
        gain_c = np.broadcast_to(
            q_gain[g * HL : (g + 1) * HL][None, :], (128, HL)
        ).astype(np.float32).copy()
        in_maps.append(
            {
                "xT": xTs[b],
                "wq": wq_c,
                "wkv": wkv_c,
                "wp": wpT,
                "cost": cos,
                "sint": sin,
                "gain": gain_c,
                "trimask": tri,
                "ident": ident,
            }
        )
    return in_maps


_NC_CACHE = {}
NPACK = 1  # number of AllToAll collectives the heads are split over


def run(x, Wq, Wk, Wv, Wproj, q_gain, T=None, use_collective=True, **spmd_kwargs):
    T = T if T is not None else x.shape[1]
    key = (T, use_collective)
    if key not in _NC_CACHE:
        _NC_CACHE[key] = build_nc(T, use_collective, npack=NPACK)
    nc = _NC_CACHE[key]
    in_maps = make_in_maps(x, Wq, Wk, Wv, Wproj, q_gain, T)
    res = run_bass_kernel_spmd(nc, in_maps, core_ids=list(range(NCORES)), **spmd_kwargs)
    TSL = T // NCORES
    out = np.empty((x.shape[0], T, DIM), dtype=np.float32)
    for c in range(NCORES):
        out[:, c * TSL : (c + 1) * TSL, :] = res.results[c]["out"]
    return out, res


def kernel(x, Wq, Wk, Wv, Wproj, q_gain):
    x = np.asarray(x, dtype=np.float32)
    out, _ = run(
        x,
        np.asarray(Wq, dtype=np.float32),
        np.asarray(Wk, dtype=np.float32),
        np.asarray(Wv, dtype=np.float32),
        np.asarray(Wproj, dtype=np.float32),
        np.asarray(q_gain, dtype=np.float32),
    )
    return out



# revision 19
# speedup vs baseline: 5.5832x; 1.4561x over previous
"""Trainium2 Bass kernel for GQA causal self-attention (RMS-norm QK + NTK RoPE + proj).

Sharding: 8 cores = 2 batches x 4 KV-head groups. Each core computes QKV
projections (bf16 matmuls, f32 accumulate), RMS-norm + NTK RoPE + gain, and
causal attention in a transposed-softmax formulation (scores are bounded since
q/k are RMS-normalized, so no max subtraction; softmax row sums come from a
ones-column matmul) for its (batch, kv-group). A single 8-way AllToAll then
redistributes attention outputs so each core holds full features for a disjoint
T/8-row slice of BOTH batches, and computes that slice of the output projection
(Wproj in bf16). Host only transposes/slices/concats - all FLOPs are on device.
"""

import sys

for _p in ("/opt/trn_rl_repo", "/root/.axon_site/_ro/trn_rl_repo"):
    if _p not in sys.path:
        sys.path.append(_p)

import numpy as np
import ml_dtypes

import concourse.bass as bass
import concourse.mybir as mybir
import concourse.tile as tile
from concourse import bacc
from concourse.bass import ts, ds
from concourse.bass_utils import run_bass_kernel_spmd

FP32 = mybir.dt.float32
BF16 = mybir.dt.bfloat16
AF = mybir.ActivationFunctionType
OP = mybir.AluOpType

B, DIM, H, HKV = 2, 2048, 16, 4
D = 128
HALF = D // 2
G = H // HKV  # q heads per kv head (= heads per core)
HL = G  # 4 local q heads
ROPE_BASE = 10000.0
TRAIN_LEN = 1024
EPS = float(np.finfo(np.float32).eps)
NCORES = 8
GRP = 4  # cores per batch group


def build_nc(
    T: int,
    use_collective: bool = True,
    phases: int = 4,
    p1stop: int = 60,
    packed_coll: bool = False,
    npack: int | None = None,
) -> bass.Bass:
    """Build the per-core Bass program (same program on all 8 cores).

    npack: number of AllToAll collectives the 4 heads are split over
    (1 = single packed exchange, 4 = one per head). Each fires as soon
    as its head group's attention output is staged.
    """
    if npack is None:
        npack = 1 if packed_coll else HL
    GH = HL // npack  # heads per collective group
    TB = T // 128  # t-blocks
    CH = min(512, T)  # tq chunk width for attention
    NCH = T // CH
    SUB = CH // 128  # 128-blocks per chunk
    TSL = T // NCORES  # sequence slice (per batch) per core after AllToAll
    MT = min(128, TSL)  # proj output t-block partition size
    NTB4 = TSL // MT
    OCH = 512  # proj output column chunk
    NOCH = DIM // OCH
    FO = DIM // 128  # feature k-tiles
    ISQ = 1.0 / float(np.sqrt(D))

    nc = bacc.Bacc("TRN2", target_bir_lowering=False, debug=False, num_devices=NCORES)

    xT = nc.dram_tensor("xT", [DIM, T], BF16, kind="ExternalInput")
    wq = nc.dram_tensor("wq", [DIM, HL * D], BF16, kind="ExternalInput")
    wkv = nc.dram_tensor("wkv", [DIM, 2 * D], BF16, kind="ExternalInput")
    wp = nc.dram_tensor("wp", [DIM, DIM], BF16, kind="ExternalInput")
    cost = nc.dram_tensor("cost", [T, HALF], FP32, kind="ExternalInput")
    sint = nc.dram_tensor("sint", [T, HALF], FP32, kind="ExternalInput")
    gain = nc.dram_tensor("gain", [128, HL], FP32, kind="ExternalInput")
    trimask = nc.dram_tensor("trimask", [128, 128], BF16, kind="ExternalInput")
    ident = nc.dram_tensor("ident", [128, 128], FP32, kind="ExternalInput")
    out = nc.dram_tensor("out", [B, TSL, DIM], FP32, kind="ExternalOutput")

    rg = [list(range(NCORES))]

    with tile.TileContext(nc) as tc:
        with (
            tc.tile_pool(name="consts", bufs=1) as consts,
            tc.tile_pool(name="persist", bufs=1) as persist,
            tc.tile_pool(name="dram", bufs=1, space="DRAM") as dram,
        ):
            # ---- resident constants / weights ----
            # weights on the sync queue (needed first); small consts on the
            # scalar queue so the first xt load isn't stuck behind them
            wq_sb = consts.tile([128, FO, HL * D], BF16)
            nc.sync.dma_start(wq_sb[:], wq.ap().rearrange("(fo fi) h -> fi fo h", fi=128))
            wkv_sb = consts.tile([128, FO, 2 * D], BF16)
            nc.sync.dma_start(wkv_sb[:], wkv.ap().rearrange("(fo fi) h -> fi fo h", fi=128))
            cos_sb = consts.tile([128, TB, HALF], FP32)
            nc.scalar.dma_start(cos_sb[:], cost.ap().rearrange("(tb p) h -> p tb h", p=128))
            sin_sb = consts.tile([128, TB, HALF], FP32)
            nc.scalar.dma_start(sin_sb[:], sint.ap().rearrange("(tb p) h -> p tb h", p=128))
            gain_sb = consts.tile([128, HL], FP32)
            nc.scalar.dma_start(gain_sb[:], gain.ap())
            tri_sb = consts.tile([128, 128], BF16)
            nc.scalar.dma_start(tri_sb[:], trimask.ap())
            id_sb = consts.tile([128, 128], FP32)
            nc.scalar.dma_start(id_sb[:], ident.ap())
            ones_sb = consts.tile([128, 1], BF16)
            nc.vector.memset(ones_sb[:], 1.0)
            eps_sb = consts.tile([128, 1], FP32)
            nc.vector.memset(eps_sb[:], EPS)

            # ---- persistent activation buffers ----
            qT_sb = persist.tile([128, HL, T], BF16)  # q, d-major per head
            kT_sb = persist.tile([128, T], BF16)  # k, d-major
            v_sb = persist.tile([128, TB, D], BF16)  # v, t-major tiles

            # one exchange buffer pair per head-group collective
            a2a_gin = [
                dram.tile([NCORES, GH, D, TSL], BF16, name=f"a2a_in{p}")
                for p in range(npack)
            ]
            a2a_gout = [
                dram.tile([NCORES, GH, D, TSL], BF16, name=f"a2a_out{p}")
                for p in range(npack)
            ]
            a2a_in = [a2a_gin[h // GH][:, h % GH] for h in range(HL)]
            a2a_out = [a2a_gout[h // GH][:, h % GH] for h in range(HL)]

            # =============== Phase 1: QKV + norm + rope + transpose ===============
            with (
                tc.tile_pool(name="p1sb", bufs=2) as p1sb,
                tc.tile_pool(name="p1ps", bufs=2, space="PSUM") as p1ps,
                tc.tile_pool(name="p1tp", bufs=2, space="PSUM") as p1tp,
            ):
                kt_ps = None
                for tb in range(TB):
                    xt = p1sb.tile([128, FO, 128], BF16, tag="xt")
                    nc.sync.dma_start(
                        xt[:], xT.ap().rearrange("(fo fi) t -> fi fo t", fi=128)[:, :, ts(tb, 128)]
                    )
                    q_ps = p1ps.tile([128, HL * D], FP32, tag="q_ps")
                    kv_ps = p1ps.tile([128, 2 * D], FP32, tag="kv_ps")
                    for fo in range(FO):
                        nc.tensor.matmul(
                            q_ps[:], xt[:, fo, :], wq_sb[:, fo, :],
                            start=(fo == 0), stop=(fo == FO - 1),
                        )
                    for fo in range(FO):
                        nc.tensor.matmul(
                            kv_ps[:], xt[:, fo, :], wkv_sb[:, fo, :],
                            start=(fo == 0), stop=(fo == FO - 1),
                        )

                    # stage to SBUF (DVE); squares read PSUM directly (ACT)
                    q_sb = p1sb.tile([128, HL * D], FP32, tag="q_sb")
                    nc.vector.tensor_copy(q_sb[:], q_ps[:])
                    k_sb = p1sb.tile([128, D], FP32, tag="k_sb")
                    nc.vector.tensor_copy(k_sb[:], kv_ps[:, :D])
                    nc.vector.tensor_copy(v_sb[:, tb, :], kv_ps[:, D:])

                    # rms stats: rms = sqrt(sumsq/D + eps) per head (q: 0..3, k: 4)
                    sumsq = p1sb.tile([128, HL + 1], FP32, tag="sumsq")
                    scrq = p1sb.tile([128, HL * D], FP32, tag="scrq")
                    nc.scalar.square(scrq[:], q_ps[:])
                    scrk = p1sb.tile([128, D], FP32, tag="scrk")
                    nc.scalar.square(scrk[:], kv_ps[:, :D])
                    nc.vector.tensor_reduce(
                        sumsq[:, :HL],
                        scrq[:].rearrange("p (h d) -> p h d", d=D),
                        mybir.AxisListType.X, OP.add,
                    )
                    nc.vector.tensor_reduce(
                        sumsq[:, HL : HL + 1], scrk[:], mybir.AxisListType.X, OP.add
                    )
                    rms = p1sb.tile([128, HL + 1], FP32, tag="rms")
                    nc.scalar.activation(
                        rms[:], sumsq[:], AF.Sqrt, bias=eps_sb[:], scale=1.0 / D
                    )
                    scl = p1sb.tile([128, HL + 1], FP32, tag="scl")
                    nc.vector.reciprocal(scl[:], rms[:])
                    sclg = p1sb.tile([128, HL], FP32, tag="sclg")
                    nc.vector.tensor_mul(sclg[:], scl[:, :HL], gain_sb[:])

                    # fold rms scale (and gain) into per-head cos/sin tables, so
                    # the rope output is already final — no post-scale pass
                    csg = p1sb.tile([128, HL, HALF], FP32, tag="csg")
                    ssg = p1sb.tile([128, HL, HALF], FP32, tag="ssg")
                    for h in range(HL):
                        nc.gpsimd.tensor_scalar_mul(
                            csg[:, h, :], cos_sb[:, tb, :], sclg[:, h : h + 1]
                        )
                        nc.gpsimd.tensor_scalar_mul(
                            ssg[:, h, :], sin_sb[:, tb, :], sclg[:, h : h + 1]
                        )
                    q4 = q_sb[:].rearrange("p (h two half) -> p h two half", two=2, half=HALF)
                    cos_b = csg[:, :, None, :].to_broadcast([128, HL, 2, HALF])
                    sin_b = ssg[:, :, None, :].to_broadcast([128, HL, 2, HALF])
                    qa = p1sb.tile([128, HL, 2, HALF], FP32, tag="qa")
                    qb = p1sb.tile([128, HL, 2, HALF], FP32, tag="qb")
                    nc.vector.tensor_mul(qa[:], q4, cos_b)
                    nc.vector.tensor_mul(qb[:], q4, sin_b)
                    q_fin = p1sb.tile([128, HL, 2, HALF], FP32, tag="q_fin")
                    nc.vector.tensor_add(q_fin[:, :, 0, :], qa[:, :, 0, :], qb[:, :, 1, :])
                    nc.vector.tensor_sub(q_fin[:, :, 1, :], qa[:, :, 1, :], qb[:, :, 0, :])

                    ck = p1sb.tile([128, HALF], FP32, tag="ck")
                    sk = p1sb.tile([128, HALF], FP32, tag="sk")
                    nc.gpsimd.tensor_scalar_mul(ck[:], cos_sb[:, tb, :], scl[:, HL : HL + 1])
                    nc.gpsimd.tensor_scalar_mul(sk[:], sin_sb[:, tb, :], scl[:, HL : HL + 1])
                    k2 = k_sb[:].rearrange("p (two half) -> p two half", two=2)
                    cos_k = ck[:, None, :].to_broadcast([128, 2, HALF])
                    sin_k = sk[:, None, :].to_broadcast([128, 2, HALF])
                    ka = p1sb.tile([128, 2, HALF], FP32, tag="ka")
                    kb = p1sb.tile([128, 2, HALF], FP32, tag="kb")
                    nc.gpsimd.tensor_mul(ka[:], k2, cos_k)
                    nc.gpsimd.tensor_mul(kb[:], k2, sin_k)
                    k_fin = p1sb.tile([128, 2, HALF], FP32, tag="k_fin")
                    nc.gpsimd.tensor_add(k_fin[:, 0, :], ka[:, 0, :], kb[:, 1, :])
                    nc.gpsimd.tensor_sub(k_fin[:, 1, :], ka[:, 1, :], kb[:, 0, :])

                    # transpose to d-major
                    qf2 = q_fin[:].rearrange("p h two half -> p (h two half)")
                    kf2 = k_fin[:].rearrange("p two half -> p (two half)")
                    qt_ps = p1tp.tile([128, HL * D], FP32, tag="qt_ps")
                    for h in range(HL):
                        nc.tensor.transpose(qt_ps[:, ts(h, D)], qf2[:, ts(h, D)], id_sb[:])
                    nc.vector.tensor_copy(
                        qT_sb[:, :, ts(tb, 128)],
                        qt_ps[:].rearrange("p (h t) -> p h t", h=HL),
                    )
                    if tb % 4 == 0:
                        kt_ps = p1tp.tile([128, 4 * D], FP32, tag="kt_ps")
                    nc.tensor.transpose(kt_ps[:, ts(tb % 4, D)], kf2[:], id_sb[:])
                    if tb % 4 == 3 or tb == TB - 1:
                        nb = tb % 4 + 1
                        nc.vector.tensor_copy(
                            kT_sb[:, ds((tb - nb + 1) * 128, nb * 128)], kt_ps[:, : nb * 128]
                        )

            if phases <= 1:
                # debug: dump v (and qT when built) so phase 1 stays live
                nelem = B * TSL * DIM
                with tc.tile_pool(name="dbg", bufs=1) as dbg:
                    dt = dbg.tile([128, nelem // 128], FP32)
                    nc.vector.memset(dt[:], 0.0)
                    nc.vector.tensor_copy(
                        dt[:, : TB * D], v_sb[:].rearrange("p tb d -> p (tb d)")
                    )
                    if p1stop >= 60:
                        nc.vector.tensor_copy(
                            dt[:], qT_sb[:].rearrange("p h t -> p (h t)")[:, : nelem // 128]
                        )
                    nc.sync.dma_start(
                        out.ap()
                        .rearrange("b t o -> (b t o)")
                        .rearrange("(p f) -> p f", p=128),
                        dt[:],
                    )

            # =============== Phase 2: causal attention (transposed softmax) ========
            with (
                tc.tile_pool(name="p2sb", bufs=6) as p2sb,
                tc.tile_pool(name="p2sp", bufs=4, space="PSUM") as p2sp,
                tc.tile_pool(name="p2op", bufs=2, space="PSUM") as p2op,
                tc.tile_pool(name="p2mp", bufs=2, space="PSUM") as p2mp,
            ):
                for h in range(HL if phases >= 2 else 0):
                    for c in range(NCH):
                        nblk = SUB * c + SUB  # total tk blocks for this chunk
                        o_ps = p2op.tile([128, CH], FP32, tag="o_ps")
                        sum_ps = p2mp.tile([1, CH], FP32, tag="sum_ps")
                        for j in range(nblk):
                            dj = j - SUB * c  # >= 0 on diagonal blocks
                            off = 0 if dj < 0 else dj * 128
                            w = CH - off
                            pT = p2sb.tile([128, CH], BF16, tag="pT")
                            s_ps = p2sp.tile([128, CH], FP32, tag="s_ps")
                            nc.tensor.matmul(
                                s_ps[:, off:CH], kT_sb[:, ts(j, 128)],
                                qT_sb[:, h, ds(c * CH + off, w)],
                                start=True, stop=True,
                            )
                            nc.scalar.activation(
                                pT[:, off:CH], s_ps[:, off:CH], AF.Exp, scale=ISQ
                            )
                            if dj >= 0:
                                nc.vector.tensor_mul(
                                    pT[:, off : off + 128], pT[:, off : off + 128], tri_sb[:]
                                )
                            # columns [:off] of pT are stale garbage from the
                            # recycled slot; o/sum matmuls only read [off:], so
                            # no memset is needed (j==0 is always full width).
                            nc.tensor.matmul(
                                o_ps[:, off:CH], v_sb[:, j, :], pT[:, off:CH],
                                start=(j == 0), stop=(j == nblk - 1),
                            )
                            nc.tensor.matmul(
                                sum_ps[:, off:CH], ones_sb[:], pT[:, off:CH],
                                start=(j == 0), stop=(j == nblk - 1),
                            )
                        rs = p2sb.tile([1, CH], FP32, tag="rs")
                        nc.vector.reciprocal(rs[:], sum_ps[:])
                        rb = p2sb.tile([128, CH], FP32, tag="rb")
                        nc.gpsimd.partition_broadcast(rb[:], rs[:])
                        stage = p2sb.tile([128, CH], BF16, tag="stage")
                        nc.vector.tensor_mul(stage[:], o_ps[:], rb[:])
                        nsl = CH // TSL
                        for sl in range(nsl):
                            nc.sync.dma_start(
                                a2a_in[h][c * nsl + sl, :, :],
                                stage[:, ts(sl, TSL)],
                            )
                    # Phase 3 (interleaved): fire a head group's AllToAll as
                    # soon as its last head is staged, overlapping the rest.
                    if phases >= 3 and use_collective and h % GH == GH - 1:
                        p = h // GH
                        nc.gpsimd.collective_compute(
                            "AllToAll", OP.bypass, replica_groups=rg,
                            ins=[a2a_gin[p][:]], outs=[a2a_gout[p][:]],
                        )

            if phases >= 3 and not use_collective:
                for p in range(npack):
                    nc.sync.dma_start(a2a_gout[p][:], a2a_gin[p][:])

            # =============== Phase 4: output projection (row-sharded) =============
            with (
                tc.tile_pool(name="p4sb", bufs=2) as p4sb,
                tc.tile_pool(name="p4in", bufs=1) as p4in,
                tc.tile_pool(name="p4ps", bufs=4, space="PSUM") as p4ps,
            ):
                pin_sb = p4in.tile([128, B, FO, TSL], BF16)
                # load + accumulate in head-major order so proj matmuls can
                # begin once head 0's AllToAll has landed
                fo_order = [g * HL + h for h in range(HL) for g in range(GRP)]
                for h in range(HL if phases >= 4 else 0):
                    for beta in range(B):
                        for g in range(GRP):
                            nc.gpsimd.dma_start(
                                pin_sb[:, beta, g * HL + h],
                                a2a_out[h][beta * GRP + g],
                            )
                for oc in range(NOCH if phases >= 4 else 0):
                    wp_sb = p4sb.tile([128, FO, OCH], BF16, tag="wp_sb")
                    nc.sync.dma_start(
                        wp_sb[:],
                        wp.ap().rearrange("(fo fi) o -> fi fo o", fi=128)[:, :, ts(oc, OCH)],
                    )
                    for beta in range(B):
                        for tb in range(NTB4):
                            pr_ps = p4ps.tile([MT, OCH], FP32, tag="pr_ps")
                            for i, fo in enumerate(fo_order):
                                nc.tensor.matmul(
                                    pr_ps[:], pin_sb[:, beta, fo, ts(tb, MT)],
                                    wp_sb[:, fo, :],
                                    start=(i == 0), stop=(i == FO - 1),
                                )
                            o_sb = p4sb.tile([MT, OCH], FP32, tag="o_sb")
                            nc.vector.tensor_copy(o_sb[:], pr_ps[:])
                            nc.scalar.dma_start(
                                out.ap()[beta, ts(tb, MT), ts(oc, OCH)], o_sb[:]
                            )

    nc.compile()
    return nc


def _rope_tables(T: int):
    if T > TRAIN_LEN:
        scale = T / TRAIN_LEN
        base = ROPE_BASE * scale ** (D / (D - 2))
    else:
        base = ROPE_BASE
    inv_freq = 1.0 / base ** (np.arange(0, D, 2, dtype=np.float32) / D)
    freqs = np.outer(np.arange(T, dtype=np.float32), inv_freq)
    return (
        np.cos(freqs).astype(np.float32),
        np.sin(freqs).astype(np.float32),
    )


def make_in_maps(x, Wq, Wk, Wv, Wproj, q_gain, T: int):
    cos, sin = _rope_tables(T)
    tri = np.triu(np.ones((128, 128), dtype=ml_dtypes.bfloat16))
    ident = np.eye(128, dtype=np.float32)
    wpT = np.ascontiguousarray(Wproj.T).astype(ml_dtypes.bfloat16)
    xTs = [np.ascontiguousarray(x[b].T).astype(ml_dtypes.bfloat16) for b in range(x.shape[0])]
    in_maps = []
    for c in range(NCORES):
        b, g = c // GRP, c % GRP
        wq_c = np.ascontiguousarray(Wq[g * HL * D : (g + 1) * HL * D, :].T).astype(
            ml_dtypes.bfloat16
        )
        wkv_c = np.ascontiguousarray(
            np.concatenate([Wk[g * D : (g + 1) * D, :], Wv[g * D : (g + 1) * D, :]], axis=0).T
        ).astype(ml_dtypes.bfloat16)
        gain_c = np.broadcast_to(
            q_gain[g * HL : (g + 1) * HL][None, :], (128, HL)
        ).astype(np.float32).copy()
        in_maps.append(
            {
                "xT": xTs[b],
                "wq": wq_c,
                "wkv": wkv_c,
                "wp": wpT,
                "cost": cos,
                "sint": sin,
                "gain": gain_c,
                "trimask": tri,
                "ident": ident,
            }
        )
    return in_maps


_NC_CACHE = {}
NPACK = 1  # number of AllToAll collectives the heads are split over


def run(x, Wq, Wk, Wv, Wproj, q_gain, T=None, use_collective=True, **spmd_kwargs):
    T = T if T is not None else x.shape[1]
    key = (T, use_collective)
    if key not in _NC_CACHE:
        _NC_CACHE[key] = build_nc(T, use_collective, npack=NPACK)
    nc = _NC_CACHE[key]
    in_maps = make_in_maps(x, Wq, Wk, Wv, Wproj, q_gain, T)
    res = run_bass_kernel_spmd(nc, in_maps, core_ids=list(range(NCORES)), **spmd_kwargs)
    TSL = T // NCORES
    out = np.empty((x.shape[0], T, DIM), dtype=np.float32)
    for c in range(NCORES):
        out[:, c * TSL : (c + 1) * TSL, :] = res.results[c]["out"]
    return out, res


def kernel(x, Wq, Wk, Wv, Wproj, q_gain):
    x = np.asarray(x, dtype=np.float32)
    out, _ = run(
        x,
        np.asarray(Wq, dtype=np.float32),
        np.asarray(Wk, dtype=np.float32),
        np.asarray(Wv, dtype=np.float32),
        np.asarray(Wproj, dtype=np.float32),
        np.asarray(q_gain, dtype=np.float32),
    )
    return out

